# revision 1
# baseline (speedup 1.0000x reference)
"""Trainium2 Bass kernel for nn_CrossAttention (8-head causal attention,
7 'series' heads from keys/values + 1 'cross' head from keysT/valuesT).

Strategy:
  - Host: gather heads into computation order [SERIES..., CROSS] so every
    core runs 8 identical causal-attention heads (L=1024, E=D=64).
  - Shard data-parallel over batch B=8 across the 8 NeuronCores.
  - Per core: scores computed TRANSPOSED ([s, lq] layout) so softmax
    normalization needs no on-chip transpose of P:
      scoresT = K @ Q^T  (contraction over E via pre-transposed Q/K strips)
      expT    = exp(scale * (scoresT + causal_maskT))   (ACT, bf16 out)
      out     = expT^T @ [V | 1]   (per lq-block, PSUM accumulate over s)
      out    /= Z (last column)    (DVE reciprocal + tensor_scalar mul)
  - Causal mask applied additively in PSUM via a [128,128] maskT @ I matmul
    on diagonal blocks; fully-masked blocks are never computed.
  - Software-pipelined per head-pair: loads/transposes of pair p+1 overlap
    QK/exp of pair p; AV matmuls issue one exp-group behind QK so the
    tail after the last exp is minimal.
"""

import sys

sys.path.insert(0, "/opt/trn_rl_repo")

from contextlib import ExitStack

import numpy as np

import concourse.bass as bass
import concourse.bacc as bacc
import concourse.mybir as mybir
from concourse.masks import make_causal_mask, make_identity
from concourse.tile import TileContext
from concourse.bass_utils import run_bass_kernel_spmd

F32 = mybir.dt.float32
F32R = mybir.dt.float32r
BF16 = mybir.dt.bfloat16
EXP = mybir.ActivationFunctionType.Exp

B, L, H, E = 8, 1024, 8, 64
NB = L // 128  # 8 row-blocks
SCALE = 1.0 / np.sqrt(E)  # 0.125

# np.random.RandomState(0).permutation(8) = [6 2 1 7 3 0 5 4]
SERIES = [2, 1, 7, 3, 0, 5, 4]
CROSS = 6
ORDER = SERIES + [CROSS]

# scoresT strip j covers s in [128j, 128j+128), lq in [START[j], 1024).
STARTS = [0, 128, 256, 384, 512, 640, 768, 896]
WIDTHS = [1024 - s for s in STARTS]  # [1024,896,768,640,512,384,256,256]
OFFS = np.cumsum([0] + WIDTHS).tolist()  # offsets into the expT tile
TOT = OFFS[-1]  # 4736
DIAG = [128 * j - STARTS[j] for j in range(8)]  # strip-local diagonal col
# ACT instruction groups (strips exp'd together; must be contiguous js)
ACT_GROUPS = [[0], [1], [2], [3], [4, 5], [6, 7]]


def build_nc():
    nc = bacc.Bacc("TRN2")
    q = nc.dram_tensor("q", [L, H * E], F32, kind="ExternalInput")
    k = nc.dram_tensor("k", [L, H * E], F32, kind="ExternalInput")
    v = nc.dram_tensor("v", [L, H * E], F32, kind="ExternalInput")
    o = nc.dram_tensor("o", [L, H * E], F32, kind="ExternalOutput")

    # [128, NB, 512] views: partition = l within block, i = l-block
    q_r = q.rearrange("(i p) c -> p i c", p=128)
    k_r = k.rearrange("(i p) c -> p i c", p=128)
    v_r = v.rearrange("(i p) c -> p i c", p=128)
    o_r = o.rearrange("(i p) c -> p i c", p=128)

    # groups of (strip j, strip-local lo, hi): one PSUM tile + one ACT each
    GROUPS0 = [
        [(0, 0, 512)],
        [(0, 512, 1024)],
        [(1, 0, 896), (2, 0, 512)],
        [(2, 512, 768), (3, 0, 640)],
        [(4, 0, 512), (5, 0, 384), (6, 0, 256), (7, 0, 128)],
    ]
    GROUPS = [
        [(0, 0, 1024)],
        [(1, 0, 896), (2, 0, 512)],
        [(2, 512, 768), (3, 0, 640)],
        [(4, 0, 512), (5, 0, 384), (6, 0, 256), (7, 0, 128)],
    ]
    GROUPS3 = [
        [(0, 0, 1024)],
        [(1, 0, 896), (2, 0, 512)],
        [(2, 512, 768), (3, 0, 640)],
        [(4, 0, 512), (5, 0, 384)],
        [(6, 0, 256), (7, 0, 128)],
    ]

    with TileContext(nc) as tc, ExitStack() as ctx:
        consts = ctx.enter_context(tc.tile_pool(name="consts", bufs=1))
        strips = ctx.enter_context(tc.tile_pool(name="strips", bufs=1))
        vab = ctx.enter_context(tc.tile_pool(name="vab", bufs=1))
        stage = ctx.enter_context(tc.tile_pool(name="stage", bufs=4))
        # shared PSUM pool: transposes + score groups (tiles <= [128,1024])
        scp = ctx.enter_context(tc.tile_pool(name="scp", bufs=2, space="PSUM"))
        avp = ctx.enter_context(tc.tile_pool(name="avp", bufs=1, space="PSUM"))
        epi = ctx.enter_context(tc.tile_pool(name="epi", bufs=2))
        expp = ctx.enter_context(tc.tile_pool(name="expp", bufs=2))

        ident = consts.tile([128, 128], BF16)
        make_identity(nc, ident)
        maskT = consts.tile([128, 128], BF16)
        make_causal_mask(nc, maskT, mask_val=-1e4)
        idbf = consts.tile([128, 128], BF16)
        make_identity(nc, idbf)

        # transposed strips, split in 512-col (l) halves so consumers start early
        qth = [[strips.tile([128, 512], BF16, tag=f"qt{p}{hf}", name=f"qt{p}{hf}")
                for hf in range(2)] for p in range(4)]
        kth = [[strips.tile([128, 512], BF16, tag=f"kt{p}{hf}", name=f"kt{p}{hf}")
                for hf in range(2)] for p in range(4)]

        def load_pair0(hf):
            """Startup: interleaved q/k half-loads so the first QK fires early."""
            for src_r, dsts in ((q_r, qth[0]), (k_r, kth[0])):
                st = stage.tile([128, 4, 128], F32, tag="sth", name="sth")
                nc.sync.dma_start(out=st, in_=src_r[:, 4 * hf : 4 * hf + 4, 0:128])
                stb = stage.tile([128, 4, 128], BF16, tag="stb", name="stb")
                nc.vector.tensor_copy(stb, st)
                ps = scp.tile([128, 512], BF16, tag="sc", name="tps0")
                for c in range(4):
                    nc.tensor.transpose(ps[:, 128 * c : 128 * (c + 1)], stb[:, c, :], ident)
                nc.vector.tensor_copy(dsts[hf], ps)

        def load_tensor(p, src_r, dsts):
            """DMA pair-p columns of q/k, convert bf16, transpose, write strips."""
            st = stage.tile([128, NB, 128], F32, tag="st", name="st")
            nc.sync.dma_start(out=st, in_=src_r[:, :, 128 * p : 128 * (p + 1)])
            ps = scp.tile([128, 1024], BF16, tag="sc", name="tps")
            for hf in range(2):
                stb = stage.tile([128, 4, 128], BF16, tag="stb", name="stb")
                nc.gpsimd.tensor_copy(stb, st[:, 4 * hf : 4 * hf + 4, :])
                for c in range(4):
                    i = 4 * hf + c
                    nc.tensor.transpose(
                        ps[:, 128 * i : 128 * (i + 1)], stb[:, c, :], ident
                    )
                nc.vector.tensor_copy(dsts[hf], ps[:, 512 * hf : 512 * (hf + 1)])

        def load_v():
            va = []
            for vh in range(2):
                st = stage.tile([128, 4, 512], F32, tag=f"stv{vh}", name=f"stv{vh}")
                nc.gpsimd.dma_start(out=st, in_=v_r[:, 4 * vh : 4 * vh + 4, :])
                for jj in range(4):
                    j = 4 * vh + jj
                    t = vab.tile([128, 520], BF16, tag=f"va{j}", name=f"va{j}")
                    tr = t.rearrange("p (h c) -> p h c", c=65)
                    nc.gpsimd.memset(tr[:, :, 64:65], 1.0)
                    nc.gpsimd.tensor_copy(
                        tr[:, :, 0:64],
                        st[:, jj, :].rearrange("p (h c) -> p h c", c=64),
                    )
                    va.append(t)
            return va

        expt = {}
        avps = {}

        def piece_chunks(j, lo, hi, pb):
            """Split piece [lo,hi) of strip j into matmul chunks that cross
            neither a PSUM bank boundary (tile-local) nor an lq half
            boundary (qt half tiles). pb = tile-local base of the piece."""
            bounds = {lo, hi}
            for b in range(512, 2048, 512):  # tile-local bank bounds
                x = b - pb + lo
                if lo < x < hi:
                    bounds.add(x)
            for lq in (512,):  # qt half boundary in lq space
                x = lq - STARTS[j]
                if lo < x < hi:
                    bounds.add(x)
            bb = sorted(bounds)
            return list(zip(bb, bb[1:]))

        def qk_group(ha, hb, pieces):
            """QK matmuls + causal mask + exp for one ACT group of a pair."""
            gw = sum(hi - lo for _, lo, hi in pieces)
            if pieces is GROUPS0[0] or pieces is GROUPS0[1]:
                pt = {
                    h: avp.tile([128, gw], F32, tag=f"av{z}", name=f"sc{h}")
                    for z, h in enumerate((ha, hb))
                }
            else:
                pt = {
                    h: scp.tile([128, gw], F32, tag="sc", name=f"sc{h}")
                    for h in (ha, hb)
                }
            # build op sequence (same for both heads) with tile-local spans
            seq = []  # (kind, j, c0, c1, t0, t1)
            pb = 0
            for j, lo, hi in pieces:
                d0 = DIAG[j]
                for c0, c1 in piece_chunks(j, lo, hi, pb):
                    seq.append(("qk", j, c0, c1, pb + c0 - lo, pb + c1 - lo))
                if lo <= d0 < hi:
                    seq.append(("mask", j, d0, d0 + 128, pb + d0 - lo, pb + d0 - lo + 128))
                pb += hi - lo
            # bank-granular start/stop: first/last op touching each psum bank
            first = {}
            last = {}
            for idx, (_, _, _, _, t0, _) in enumerate(seq):
                b = t0 // 512
                first.setdefault(b, idx)
                last[b] = idx
            heads_outer = pieces is GROUPS0[0]
            iters = (
                [(h, x) for h in (ha, hb) for x in enumerate(seq)]
                if heads_outer
                else [(h, x) for x in enumerate(seq) for h in (ha, hb)]
            )
            for h, (idx, (kind, j, c0, c1, t0, t1)) in iters:
                b = t0 // 512
                st, sp = first[b] == idx, last[b] == idx
                if True:
                    p, po = h // 2, 64 * (h % 2)
                    if kind == "qk":
                        lq0 = STARTS[j] + c0
                        nc.tensor.matmul(
                            pt[h][:, t0:t1],
                            kth[p][j // 4][po : po + 64, (128 * j) % 512 : (128 * j) % 512 + 128],
                            qth[p][lq0 // 512][po : po + 64, lq0 % 512 : lq0 % 512 + (c1 - c0)],
                            start=st,
                            stop=sp,
                        )
                    else:
                        nc.tensor.matmul(
                            pt[h][:, t0:t1], maskT, idbf, start=st, stop=sp
                        )
            for h in (ha, hb):
                j0, lo0, _ = pieces[0]
                nc.scalar.activation(
                    out=expt[h][:, OFFS[j0] + lo0 : OFFS[j0] + lo0 + gw],
                    in_=pt[h][:, :],
                    func=EXP,
                    scale=SCALE,
                )

        def av_mm(h, i, j):
            av = avps[h]
            off = OFFS[j] + 128 * i - STARTS[j]
            sl = 65 * (i % 4)
            nc.tensor.matmul(
                av[:, sl : sl + 65],
                expt[h][:, off : off + 128],
                va[j][:, 65 * h : 65 * h + 65],
                start=(j == 0 and i % 4 == 0),
                stop=(j == i and i % 4 == 3),
            )

        def av_pieces(va, ha, hb, pieces, deferred):
            """Wave-A AV matmuls for completed strips; defer wave B until the
            wave-A psum bank is epilogued and reused."""
            for h in (ha, hb):
                for j, lo, hi in pieces:
                    if hi != WIDTHS[j]:
                        continue
                    for i in range(j, NB):
                        if i < 4 or wave_b_open.get(h):
                            av_mm(h, i, j)
                        else:
                            deferred[h].append((i, j))

        def flush_wave_b(h):
            wave_b_open[h] = True
            for i, j in sorted(deferred_of[h]):
                av_mm(h, i, j)
            deferred_of[h].clear()

        outsb_of = {}

        def epilogue_wave(ha, hb, p, w):
            if w == 0:
                outsb_of[p] = epi.tile([128, NB, 128], F32, tag="osb", name="osb")
            outsb = outsb_of[p]
            i0 = 4 * w
            for z, h in enumerate((ha, hb)):
                av = avps[h]
                r4 = epi.tile([128, 4], F32, tag=f"r4{z}", name=f"r4{z}")
                nc.vector.reciprocal(
                    r4.rearrange("p (i u) -> p i u", u=1),
                    av.rearrange("p (i c) -> p i c", c=65)[:, :, 64:65],
                )
                rb = bass.AP(
                    tensor=r4.tensor, offset=r4.offset, ap=[r4.ap[0], [1, 4], [0, 64]]
                )
                nc.vector.tensor_mul(
                    outsb[:, i0 : i0 + 4, 64 * z : 64 * z + 64],
                    av.rearrange("p (i c) -> p i c", c=65)[:, :, 0:64],
                    rb,
                )
            eng = nc.sync if (p == 3 and w == 1) else nc.gpsimd
            eng.dma_start(
                out=o_r[:, i0 : i0 + 4, 128 * p : 128 * (p + 1)],
                in_=outsb[:, i0 : i0 + 4, :],
            )

        # ---- software-pipelined main loop over head pairs
        pending = []
        deferred_of = {}
        wave_b_open = {}
        load_pair0(0)
        load_pair0(1)
        va = load_v()
        for p in range(4):
            ha, hb = 2 * p, 2 * p + 1
            expt[ha] = expp.tile([128, TOT], BF16, tag="e0", name=f"e{ha}")
            expt[hb] = expp.tile([128, TOT], BF16, tag="e1", name=f"e{hb}")
            avps.pop(ha, None)
            avps.pop(hb, None)
            deferred_of[ha] = []
            deferred_of[hb] = []
            wave_b_open[ha] = False
            wave_b_open[hb] = False
            groups = GROUPS0 if p == 0 else (GROUPS3 if p == 3 else GROUPS)
            lq, lk = (3, 4) if p == 0 else (2, 3)
            prev = None
            nolag = p == 3
            for gi, pieces in enumerate(groups):
                qk_group(ha, hb, pieces)
                if gi == 0 and pending:
                    pending.pop()()
                if gi == lq and p < 3:
                    load_tensor(p + 1, q_r, qth[p + 1])
                if gi == lk and p < 3:
                    load_tensor(p + 1, k_r, kth[p + 1])
                todo = pieces if nolag else prev
                if todo is not None:
                    if ha not in avps:
                        avps[ha] = avp.tile([128, 260], F32, tag="av0", name=f"avA{ha}")
                        avps[hb] = avp.tile([128, 260], F32, tag="av1", name=f"avA{hb}")
                    av_pieces(va, ha, hb, todo, deferred_of)
                    if any(j == 3 and hi == WIDTHS[3] for j, _, hi in todo):
                        # wave A complete: epilogue its bank, reuse for wave B
                        epilogue_wave(ha, hb, p, 0)
                        avps[ha] = avp.tile([128, 260], F32, tag="av0", name=f"avB{ha}")
                        avps[hb] = avp.tile([128, 260], F32, tag="av1", name=f"avB{hb}")
                        flush_wave_b(ha)
                        flush_wave_b(hb)
                prev = pieces
            if nolag:
                epilogue_wave(ha, hb, p, 1)
            else:
                def mk(pv, a, b, pp):
                    def fin():
                        av_pieces(va, a, b, pv, deferred_of)
                        flush_wave_b(a)
                        flush_wave_b(b)
                        epilogue_wave(a, b, pp, 1)
                    return fin
                pending.append(mk(prev, ha, hb, p))
        if pending:
            pending.pop()()

    nc.finalize()
    return nc


_NC = None


def _get_nc():
    global _NC
    if _NC is None:
        _NC = build_nc()
    return _NC


def kernel(queries, keys, keysT, values, valuesT, trace=False):
    queries = np.asarray(queries, dtype=np.float32)
    keys = np.asarray(keys, dtype=np.float32)
    keysT = np.asarray(keysT, dtype=np.float32)
    values = np.asarray(values, dtype=np.float32)
    valuesT = np.asarray(valuesT, dtype=np.float32)

    qg = queries[:, :, ORDER]  # [B, L, 8, E]
    kg = np.concatenate([keys[:, :, SERIES], keysT[:, :, CROSS : CROSS + 1]], axis=2)
    vg = np.concatenate(
        [values[:, :, SERIES], valuesT[:, :, CROSS : CROSS + 1]], axis=2
    )

    in_maps = [
        {
            "q": np.ascontiguousarray(qg[b].reshape(L, H * E)),
            "k": np.ascontiguousarray(kg[b].reshape(L, H * E)),
            "v": np.ascontiguousarray(vg[b].reshape(L, H * E)),
        }
        for b in range(B)
    ]
    res = run_bass_kernel_spmd(
        _get_nc(), in_maps, core_ids=list(range(B)), trace=trace
    )
    out = np.stack([res.results[b]["o"].reshape(L, H, E) for b in range(B)])
    if trace:
        kernel.last_exec_time_ns = res.exec_time_ns
    return out


kernel.last_exec_time_ns = None

if __name__ == "__main__":
    rng = np.random.RandomState(1)
    shp = (B, L, H, E)
    ins = {
        n: rng.randn(*shp).astype(np.float32)
        for n in ("queries", "keys", "keysT", "values", "valuesT")
    }
    out = kernel(**ins)
    print("out shape", out.shape, "finite", np.isfinite(out).all())

